# revision 1
# baseline (speedup 1.0000x reference)
"""Trainium2 Bass kernel for the annular photonic transfer-matrix reflectance
sweep (W=2097152 wavelengths, L=6 layers), data-parallel over 8 NeuronCores.

Math notes (validated against the jax reference by a numpy golden model):
- Every Bessel value enters the transfer matrix as a product of one value at
  x0 and one at x1, so the sqrt(2/(pi*x)) amplitudes combine with the
  (pi/2)*x0 prefactor into the per-layer constant sqrt(r0/r1), which cancels
  in the final reflectance ratio. We therefore compute amplitude-NORMALIZED
  Bessel functions: for x>=8 the Numerical-Recipes asymptotic form directly,
  for x<8 the NR rationals rescaled by sqrt(pi*x/2).
- sin/cos at both phase offsets come from just two ACT Sin evaluations of
  A=sin(x-pi/4), B=sin(x+pi/4) after Cody-Waite range reduction (ACT Sin is
  valid only on [-pi,pi]); round-to-nearest uses the 2^23 magic-number trick.
- All divisions are exp(-ln(v)) on the ACT engine (valid: every divisor
  here is positive); custom DVE ops are not supported by this toolchain.
- mu == 1 in this problem (spec fill=ones); a general path feeds
  eps_k = eps*mu and eps_p = eps/mu as two inputs.
"""
import numpy as np

import bass_rust
import concourse.bass as bass
import concourse.tile as tile
from concourse import mybir
from concourse.vector_clock import ScopedClock

F32 = mybir.dt.float32
I32 = mybir.dt.int32
AL = mybir.AluOpType
AF = mybir.ActivationFunctionType

W = 2097152
L = 6
NCORES = 8
P = 128
WS = W // NCORES          # 262144 elements per core
FT = WS // P              # 2048 free dim per core
FC = 1024                 # chunk of free dim processed at once
NSLOTS = 48               # rotating SBUF slots for work tiles

PI = float(np.pi)
TWO_OVER_PI = 0.636619772

# ---- Cody-Waite split of 2*pi ------------------------------------------------


def _split_const(v, bits=12):
    u = np.uint32(np.float32(v).view(np.uint32))
    mask = np.uint32(0xFFFFFFFF) << np.uint32(23 - bits + 1)
    return float(np.uint32(u & mask).view(np.float32))


_2PI = 2 * PI
CW_C1 = _split_const(_2PI)
CW_C2 = _split_const(_2PI - CW_C1)
CW_C3 = float(np.float32(_2PI - CW_C1 - CW_C2))
INV_2PI = float(np.float32(1.0 / _2PI))
MAGIC = 8388608.0

# ---- NR Bessel coefficients (highest degree first) ---------------------------

J0_NUM = [-184.9052456, 77392.33017, -11214424.18, 651619640.7,
          -13362590354.0, 57568490574.0]
J0_DEN = [1.0, 267.8532712, 59272.64853, 9494680.718,
          1029532985.0, 57568490411.0]
J1_NUM = [-30.16036606, 15704.48260, -2972611.439, 242396853.1,
          -7895059235.0, 72362614232.0]
J1_DEN = [1.0, 376.9991397, 99447.43394, 18583304.74,
          2300535178.0, 144725228442.0]
Y0_NUM = [228.4622733, -86327.92757, 10879881.29, -512359803.6,
          7062834065.0, -2957821389.0]
Y0_DEN = [1.0, 226.1030244, 47447.26470, 7189466.438,
          745249964.8, 40076544269.0]
Y1_NUM = [8.511937935e4, -4.237922726e7, 7.349264551e9,
          -5.153438139e11, 1.275274390e13, -4.900604943e13]
Y1_DEN = [1.0, 3.549632885e3, 1.020426050e6, 2.245904002e8,
          3.733650367e10, 4.244419664e12, 2.499580570e14]
P0C = [0.2093887211e-6, -0.2073370639e-5, 0.2734510407e-4,
       -0.1098628627e-2, 1.0]
Q0C = [8 * c for c in [-0.934935152e-7, 0.7621095161e-6, -0.6911147651e-5,
                       0.1430488765e-3, -0.1562499995e-1]]
P1C = [-0.240337019e-6, 0.2457520174e-5, -0.3516396496e-4,
       0.183105e-2, 1.0]
Q1C = [8 * c for c in [0.105787412e-6, -0.88228987e-6, 0.8449199096e-5,
                       -0.2002690873e-3, 0.04687499995]]

# ---- amplitude/phase (M, theta) polynomial fits ------------------------------
# J0 = amp*m0(u)*cos(x-pi/4+phi0(u)), Y0 = amp*m0*sin(...), order 1 with
# (m1, phi1) and offset 3pi/4; u = 1/x, polys of degree 10 in v = A*u+B,
# fit over x in [XFIT_MIN, XFIT_MAX] to <=1.4e-7 (f32 Horner eval, vs scipy).
XFIT_MIN, XFIT_MAX = 1.30, 185.0
A_MAP = 2.618399564507349
B_MAP = -1.014153511159499
M0_C = [1.944259206288661e-05, -5.061002422204446e-05, 4.606362133960433e-05,
        4.030051986810358e-05, -0.0004091425408609736, 0.0016067357073027168,
        -0.0036989109853500526, -0.013529858602173836, 0.9921073136508916]
PH0_C = [1.8581115616861825e-05, -3.932864656273558e-06,
         -9.394630573303383e-05, 0.00022030499483529528,
         -0.0003364955819960292, -8.100004321597988e-05,
         0.004825921608789799, -0.04066722905163263, -0.04562179167891097]
M1_C = [-2.276806970806628e-05, 7.846452604242017e-05,
        -0.00012454803053409222, 6.0261291640032366e-05, 0.000652676767431646,
        -0.004232088855951652, 0.015408583904527717, 0.045123605129181545,
        1.025121683847755]
PH1_C = [-3.803054279414293e-05, 3.328020208716037e-05,
         0.00013415909827870358, -0.0004927599374944857, 0.0011926194911796454,
         -0.0007755446984584927, -0.014889899715191432, 0.12324671107155569,
         0.1376401393524587]

# ---- walrus 1-sync-wait-per-instruction workaround --------------------------
# This neuronxcc build rejects instructions carrying more than one semaphore
# wait ("Too many sync wait commands"). _split_waits moves excess waits onto
# same-engine nops inserted immediately before the over-limit instruction.
_MAXW = 1


def _split_waits(nc):
    for f in nc.m.functions:
        for bb in f.blocks:
            arr = list(bb.instructions)
            out = []
            changed = False
            for mi in arr:
                si = mi.sync_info
                waits = list(si.on_wait) if si is not None and si.on_wait else []
                if len(waits) > _MAXW:
                    changed = True
                    upd = list(si.on_update) if si is not None and si.on_update \
                        else []
                    rest = waits[_MAXW:]
                    for i in range(0, len(rest), _MAXW):
                        ev = nc.engines[mi.engine].nop()
                        cur = nc.cur_bb.bb
                        cur.instructions = [
                            x for x in cur.instructions if x.name != ev.ins.name
                        ]
                        ev.ins.sync_info = bass_rust.SyncInfo(
                            on_wait=rest[i:i + _MAXW], on_update=[])
                        out.append(ev.ins)
                    mi.sync_info = bass_rust.SyncInfo(on_wait=waits[:_MAXW],
                                                      on_update=upd)
                out.append(mi)
            if changed:
                bb.instructions = out


def _patched_drain_and_barrier(self, tick_clock, wait_clock):
    nc = self.nc
    drain_inst = nc.sync.drain()
    wait_clock.add_sem_waits(
        drain_inst.ins, ScopedClock({None: tick_clock.global_clock})
    )
    nc.all_engine_barrier()
    assert self.sems is not None
    popped = nc._tile_sem_poison_stack.pop()
    assert popped is self._sem_poison
    nc.clear_and_free_semaphores(list(self.sems.allocated().values()))
    nc.all_engine_barrier()


tile.TileContext._drain_and_barrier = _patched_drain_and_barrier


def _register_const(nc, *values):
    for v in values:
        v = float(v)
        if (F32, v) in nc.const_aps.aps:
            continue
        t = nc.alloc_sbuf_tensor(f"const-f32-{v}", [128, 1], F32)
        nc.gpsimd.memset(t.ap(), v)
        nc.const_aps.aps[(F32, v)] = t.ap()
    nc.all_engine_barrier()


# ---- kernel emitter ----------------------------------------------------------


class Emit:
    """Tiny op-emission helper: every value is a [P, FC] f32 tile from one
    rotating-slot pool."""

    def __init__(self, nc, pool, tt_eng=None):
        self.nc = nc
        self.pool = pool
        self.tt_eng = tt_eng
        self.n = 0

    def t(self, dtype=F32):
        self.n += 1
        if dtype is F32:
            return self.pool.tile([P, FC], dtype, name=f"w{self.n}", tag="w")
        return self.pool.tile([P, FC], dtype, name=f"w{self.n}", tag="wm",
                              bufs=4)

    def tt(self, a, b, op, eng=None):
        out = self.t()
        (eng or self.tt_eng or self.nc.vector).tensor_tensor(
            out[:], a[:], b[:], AL[op])
        return out

    def stt(self, a, s, b, op0="add", op1="mult", eng=None):
        out = self.t()
        (eng or self.nc.vector).scalar_tensor_tensor(
            out[:], a[:], float(s), b[:], AL[op0], AL[op1])
        return out

    def ts(self, a, s1, op0, s2=None, op1=None, eng=None, dtype=F32):
        out = self.t(dtype)
        e = eng or self.nc.vector
        if op1 is None:
            e.tensor_scalar(out[:], a[:], float(s1), None, AL[op0])
        else:
            e.tensor_scalar(out[:], a[:], float(s1), float(s2),
                            AL[op0], AL[op1])
        return out

    def act(self, a, func, bias=0.0, scale=1.0):
        out = self.t()
        self.nc.scalar.activation(out[:], a[:], func, float(bias), float(scale))
        return out

    def recip(self, a):
        # 1/a for a > 0 via exp(-ln a): two ACT ops, same table set.
        return self.act(self.act(a, AF.Ln), AF.Exp, 0.0, -1.0)

    def poly(self, y, coeffs):
        acc = self.ts(y, coeffs[0], "mult")
        for c in coeffs[1:-1]:
            acc = self.stt(acc, c, y, "add", "mult")
        return self.ts(acc, coeffs[-1], "add")

    def bessel(self, x):
        """Amplitude-normalized J0,Y0,J1,Y1 at x (all [P,FC] f32 tiles)."""
        nc = self.nc
        lnx = self.act(x, AF.Ln)
        rx = self.act(lnx, AF.Exp, 0.0, -1.0)        # 1/x
        y = self.act(x, AF.Square)
        y2 = self.act(rx, AF.Square, 0.0, 8.0)
        iamp = self.act(x, AF.Sqrt, 0.0, PI / 2)
        # range reduction: k = round(x/(2pi) - 1/8), xr = x - k*2pi
        # (2-term Cody-Waite; k*c1 exact since k < 2^6 and c1 has 12 bits)
        tr = self.ts(x, INV_2PI, "mult", -0.125, "add")
        kf = self.ts(tr, MAGIC, "add", MAGIC, "subtract")
        xr1 = self.stt(kf, -CW_C1, x, "mult", "add")
        xr = self.stt(kf, -CW_C2, xr1, "mult", "add")
        A = self.act(xr, AF.Sin, -PI / 4, 1.0)       # sin(x-pi/4)
        mb = self.ts(xr, 3 * PI / 4, "is_gt")
        wb = self.stt(mb, -2 * PI, xr, "mult", "add")
        B = self.act(wb, AF.Sin, PI / 4, 1.0)        # cos(x-pi/4)

        # big branch (x>=8), normalized
        p0 = self.poly(y2, P0C)
        q0 = self.poly(y2, Q0C)
        p1 = self.poly(y2, P1C)
        q1 = self.poly(y2, Q1C)
        zq0 = self.tt(rx, q0, "mult")
        zq1 = self.tt(rx, q1, "mult")
        j0b = self.tt(self.tt(B, p0, "mult"), self.tt(A, zq0, "mult"),
                      "subtract")
        y0b = self.tt(self.tt(A, p0, "mult"), self.tt(B, zq0, "mult"), "add")
        j1b = self.tt(self.tt(A, p1, "mult"), self.tt(B, zq1, "mult"), "add")
        y1b = self.tt(self.tt(A, zq1, "mult"), self.tt(B, p1, "mult"),
                      "subtract")

        # small branch (x<8): NR rationals, then normalize by iamp
        j0n = self.poly(y, J0_NUM)
        j0d = self.poly(y, J0_DEN)
        y0n = self.poly(y, Y0_NUM)
        y0d = self.poly(y, Y0_DEN)
        j1n = self.poly(y, J1_NUM)
        j1d = self.poly(y, J1_DEN)
        y1n = self.poly(y, Y1_NUM)
        y1d = self.poly(y, Y1_DEN)

        j0r = self.tt(j0n, self.recip(j0d), "mult")
        J0 = self.tt(j0r, iamp, "mult")
        y0r = self.tt(y0n, self.recip(y0d), "mult")
        tl = self.tt(j0r, lnx, "mult")
        Y0 = self.tt(self.stt(tl, TWO_OVER_PI, y0r, "mult", "add"),
                     iamp, "mult")
        j1r0 = self.tt(j1n, self.recip(j1d), "mult")
        j1r = self.tt(j1r0, x, "mult")
        J1 = self.tt(j1r, iamp, "mult")
        y1r0 = self.tt(y1n, self.recip(y1d), "mult")
        y1r = self.tt(y1r0, x, "mult")
        tl1 = self.tt(j1r, lnx, "mult")
        d = self.tt(tl1, rx, "subtract")
        Y1 = self.tt(self.stt(d, TWO_OVER_PI, y1r, "mult", "add"),
                     iamp, "mult")

        m = self.ts(x, 8.0, "is_ge", dtype=I32)
        for small, big in ((J0, j0b), (Y0, y0b), (J1, j1b), (Y1, y1b)):
            nc.vector.copy_predicated(small[:], m[:], big[:])
        return J0, Y0, J1, Y1

    # ---- amplitude/phase variant ----

    def mphi(self, x):
        """m0, phi0, m1, phi1 polynomial tiles at x."""
        lnx = self.act(x, AF.Ln)
        rx = self.act(lnx, AF.Exp, 0.0, -1.0)       # u = 1/x
        v = self.ts(rx, A_MAP, "mult", B_MAP, "add")
        return self.mphi_v(v)

    def mphi_v(self, v):
        m0 = self.poly(v, M0_C)
        p0 = self.poly(v, PH0_C)
        m1 = self.poly(v, M1_C)
        p1 = self.poly(v, PH1_C)
        return m0, p0, m1, p1

    def reduce_sin(self, x):
        """A = sin(x-pi/4), B = cos(x-pi/4) via Cody-Waite + ACT Sin."""
        tr = self.ts(x, INV_2PI, "mult", -0.125, "add")
        kf = self.ts(tr, MAGIC, "add", MAGIC, "subtract")
        xr1 = self.stt(kf, -CW_C1, x, "mult", "add")
        xr = self.stt(kf, -CW_C2, xr1, "mult", "add")
        A = self.act(xr, AF.Sin, -PI / 4, 1.0)
        mb = self.ts(xr, 3 * PI / 4, "is_gt")
        wb = self.stt(mb, -2 * PI, xr, "mult", "add")
        B = self.act(wb, AF.Sin, PI / 4, 1.0)
        return A, B

    def bessel_mtheta(self, x):
        """Amplitude-normalized J0,Y0,J1,Y1 via the M/theta fit."""
        m0, p0, m1, p1 = self.mphi(x)
        A, B = self.reduce_sin(x)
        sp0 = self.act(p0, AF.Sin)
        cp0 = self.act(p0, AF.Sin, PI / 2, 1.0)
        sp1 = self.act(p1, AF.Sin)
        cp1 = self.act(p1, AF.Sin, PI / 2, 1.0)
        J0 = self.tt(self.tt(self.tt(B, cp0, "mult"),
                             self.tt(A, sp0, "mult"), "subtract"), m0, "mult")
        Y0 = self.tt(self.tt(self.tt(A, cp0, "mult"),
                             self.tt(B, sp0, "mult"), "add"), m0, "mult")
        J1 = self.tt(self.tt(self.tt(A, cp1, "mult"),
                             self.tt(B, sp1, "mult"), "add"), m1, "mult")
        Y1 = self.tt(self.tt(self.tt(A, sp1, "mult"),
                             self.tt(B, cp1, "mult"), "subtract"), m1, "mult")
        return J0, Y0, J1, Y1

    def reduce_pm(self, x):
        """Reduce x >= 0 into [-pi, pi] (k = round(x/2pi))."""
        tr = self.ts(x, INV_2PI, "mult")
        kf = self.ts(tr, MAGIC, "add", MAGIC, "subtract")
        xr1 = self.stt(kf, -CW_C1, x, "mult", "add")
        return self.stt(kf, -CW_C2, xr1, "mult", "add")

    def shell_phase1(self, t_l, r0, r1):
        """DVE-heavy stage of one shell: reciprocal, 8 Horner polys, and the
        Cody-Waite reduction of D = x1-x0. 1/x0 = (1/t)*(1/r0), so one exp/ln
        reciprocal serves both points and the poly-variable map folds into a
        single tensor_scalar per point."""
        rt = self.recip(t_l)
        v0 = self.ts(rt, A_MAP / r0, "mult", B_MAP, "add")
        v1 = self.ts(rt, A_MAP / r1, "mult", B_MAP, "add")
        m0a, f0a, m1a, f1a = self.mphi_v(v0)
        m0b, f0b, m1b, f1b = self.mphi_v(v1)
        D = self.ts(t_l, float(np.float32(r1) - np.float32(r0)), "mult")
        Dr = self.reduce_pm(D)
        mC = self.ts(Dr, PI / 2, "is_gt")
        wC = self.stt(mC, -2 * PI, Dr, "mult", "add")
        return (m0a, f0a, m1a, f1a, m0b, f0b, m1b, f1b, Dr, wC)

    def shell_phase2(self, ph1, se, rp):
        """Trig + combine stage (ACT sins, Pool tensor-tensor)."""
        m0a, f0a, m1a, f1a, m0b, f0b, m1b, f1b, Dr, wC = ph1
        SD = self.act(Dr, AF.Sin)
        CD = self.act(wC, AF.Sin, PI / 2, 1.0)

        def cos_of(e):          # cos(Dr + e) = CD*cos(e) - SD*sin(e)
            s_, c_ = self.act(e, AF.Sin), self.act(e, AF.Sin, PI / 2, 1.0)
            return self.tt(self.tt(CD, c_, "mult"),
                           self.tt(SD, s_, "mult"), "subtract")

        def sin_of(e):          # sin(Dr + e) = SD*cos(e) + CD*sin(e)
            s_, c_ = self.act(e, AF.Sin), self.act(e, AF.Sin, PI / 2, 1.0)
            return self.tt(self.tt(SD, c_, "mult"),
                           self.tt(CD, s_, "mult"), "add")

        e00 = self.tt(f0b, f1a, "subtract")
        e01 = self.tt(f0b, f0a, "subtract")
        e10 = self.tt(f1b, f1a, "subtract")
        e11 = self.tt(f1b, f0a, "subtract")
        a = self.tt(self.tt(m1a, m0b, "mult"), cos_of(e00), "mult")
        b = self.tt(self.tt(self.tt(m0a, m0b, "mult"), sin_of(e01), "mult"),
                    rp, "mult")
        c = self.tt(self.tt(self.tt(m1a, m1b, "mult"), sin_of(e10), "mult"),
                    se, "mult")
        d = self.tt(self.tt(m0a, m1b, "mult"), cos_of(e11), "mult")
        return a, b, c, d

    def cfac_ph1(self, t, r):
        rt = self.recip(t)
        v = self.ts(rt, A_MAP / r, "mult", B_MAP, "add")
        return self.mphi_v(v)

    def cfac_ph2(self, ph1):
        m0, p0, m1, p1 = ph1
        d = self.tt(p1, p0, "subtract")
        rm = self.tt(m1, self.recip(m0), "mult")
        sd = self.act(d, AF.Sin)
        cd = self.act(d, AF.Sin, PI / 2, 1.0)
        cre = self.tt(rm, sd, "mult")
        cimn = self.tt(rm, cd, "mult")
        return cre, cimn

    def cfac_mtheta(self, t, r):
        """(cre, cimn) at x = t*r, with c1 = -(cre + i*cim), cimn = -cim.
        c1 = (m1/m0)*(-sin(d) + i*cos(d)), d = phi1 - phi0."""
        return self.cfac_ph2(self.cfac_ph1(t, r))


def build(rho, dual_eps, variant="mtheta"):
    """Build the per-core Bass program. rho: [L,2] floats (baked in)."""
    nc = bass.Bass()
    _register_const(nc, -PI / 4, PI / 4, PI / 2)
    om_d = nc.declare_dram_parameter("omega", [P, FT], F32, isOutput=False)
    ek_d = nc.declare_dram_parameter("epsk", [L, P, FT], F32, isOutput=False)
    ep_d = nc.declare_dram_parameter("epsp", [L, P, FT], F32, isOutput=False) \
        if dual_eps else ek_d
    out_d = nc.declare_dram_parameter("out", [P, FT], F32, isOutput=True)

    nslots = NSLOTS if variant == "delta" else 47
    with tile.TileContext(nc) as tc:
        with tc.tile_pool(name="work", bufs=nslots) as pool:
            for ci in range(FT // FC):
                sl = slice(ci * FC, (ci + 1) * FC)
                em = Emit(nc, pool,
                          tt_eng=nc.gpsimd if variant == "delta" else None)
                omega = em.t()
                nc.sync.dma_start(omega[:], om_d[:, sl])

                def load_eps(d, l):
                    e = em.t()
                    nc.sync.dma_start(e[:], d[l, :, sl])
                    return e

                # boundary arguments (t = omega*sqrt(eps)) and boundary p's
                ek0 = load_eps(ek_d, 0)
                se_k0 = em.act(ek0, AF.Sqrt)
                t0_ = em.tt(omega, se_k0, "mult")
                ek5 = load_eps(ek_d, L - 1)
                se_k5 = em.act(ek5, AF.Sqrt)
                t5_ = em.tt(omega, se_k5, "mult")
                if dual_eps:
                    p0e = em.act(load_eps(ep_d, 0), AF.Sqrt)
                    p1e = em.act(load_eps(ep_d, L - 1), AF.Sqrt)
                else:
                    p0e, p1e = se_k0, se_k5

                # shells
                A = B = C = D = None

                def shell_inputs(l):
                    ekl = load_eps(ek_d, l)
                    se_k = em.act(ekl, AF.Sqrt)
                    tl_ = em.tt(omega, se_k, "mult")
                    if dual_eps:
                        se_p = em.act(load_eps(ep_d, l), AF.Sqrt)
                    else:
                        se_p = se_k
                    rp = em.recip(se_p)
                    return tl_, se_p, rp

                def chain_update(abcd):
                    nonlocal A, B, C, D
                    a, b, c, d = abcd
                    if A is None:
                        A, B, C, D = a, b, c, d
                    else:
                        A2 = em.tt(em.tt(A, a, "mult"), em.tt(B, c, "mult"),
                                   "subtract")
                        B2 = em.tt(em.tt(A, b, "mult"), em.tt(B, d, "mult"),
                                   "add")
                        C2 = em.tt(em.tt(C, a, "mult"), em.tt(D, c, "mult"),
                                   "add")
                        D2 = em.tt(em.tt(D, d, "mult"), em.tt(C, b, "mult"),
                                   "subtract")
                        A, B, C, D = A2, B2, C2, D2

                if variant == "delta":
                    for l in range(1, L - 1):
                        tl_, se_p, rp = shell_inputs(l)
                        ph1 = em.shell_phase1(tl_, float(rho[l, 0]),
                                              float(rho[l, 1]))
                        chain_update(em.shell_phase2(ph1, se_p, rp))
                    cre0, cim0 = em.cfac_mtheta(t0_, float(rho[0, 1]))
                    cre1, cim1 = em.cfac_mtheta(t5_, float(rho[L - 1, 0]))
                else:
                    for l in range(1, L - 1):
                        tl_, se_p, rp = shell_inputs(l)
                        x0 = em.ts(tl_, float(rho[l, 0]), "mult")
                        x1 = em.ts(tl_, float(rho[l, 1]), "mult")
                        bes = em.bessel_mtheta if variant == "mtheta" \
                            else em.bessel
                        J0a, Y0a, J1a, Y1a = bes(x0)
                        J0b, Y0b, J1b, Y1b = bes(x1)
                        a = em.tt(em.tt(J1a, Y0b, "mult"),
                                  em.tt(Y1a, J0b, "mult"), "subtract")
                        b = em.tt(em.tt(em.tt(J0a, Y0b, "mult"),
                                        em.tt(Y0a, J0b, "mult"), "subtract"),
                                  rp, "mult")
                        c = em.tt(em.tt(em.tt(J1a, Y1b, "mult"),
                                        em.tt(Y1a, J1b, "mult"), "subtract"),
                                  se_p, "mult")
                        d = em.tt(em.tt(Y0a, J1b, "mult"),
                                  em.tt(J0a, Y1b, "mult"), "subtract")
                        chain_update((a, b, c, d))

                # boundary c-factors (raw: c1 = -(cre + i cim); the mtheta
                # path returns (cre, -cim), whose sign flips cancel in the
                # final |N|^2/|D|^2 assembly — verified in the golden model)
                def cfac(tz, rr):
                    if variant in ("mtheta", "delta"):
                        return em.cfac_mtheta(tz, rr)
                    xz = em.ts(tz, rr, "mult")
                    J0, Y0, J1, Y1 = em.bessel(xz)
                    dd = em.tt(em.act(J0, AF.Square),
                               em.act(Y0, AF.Square), "add")
                    rdd = em.recip(dd)
                    cre = em.tt(em.tt(em.tt(J1, J0, "mult"),
                                      em.tt(Y1, Y0, "mult"), "add"),
                                rdd, "mult")
                    cim = em.tt(em.tt(em.tt(Y1, J0, "mult"),
                                      em.tt(J1, Y0, "mult"), "subtract"),
                                rdd, "mult")
                    return cre, cim

                if variant != "delta":
                    cre0, cim0 = cfac(t0_, float(rho[0, 1]))
                    cre1, cim1 = cfac(t5_, float(rho[L - 1, 0]))

                ur0 = em.tt(p0e, cre0, "mult")
                ui0 = em.tt(p0e, cim0, "mult")
                vr0 = em.tt(p1e, cre1, "mult")
                vi0 = em.tt(p1e, cim1, "mult")
                Q = em.tt(ui0, B, "mult")
                er = em.tt(D, em.tt(ur0, B, "mult"), "add")
                T1 = em.tt(vi0, Q, "mult")
                T2 = em.tt(vr0, er, "mult")
                T3 = em.tt(vr0, Q, "mult")
                T4 = em.tt(vi0, er, "mult")
                aAr = em.tt(ur0, A, "mult")
                aAi = em.tt(ui0, A, "mult")
                b0 = em.tt(C, aAr, "subtract")
                b1 = em.tt(b0, T2, "add")
                Nr = em.tt(b1, T1, "subtract")
                Dr = em.tt(b1, T1, "add")
                c0_ = em.tt(aAi, T3, "subtract")
                Ni = em.tt(c0_, T4, "subtract")
                Di = em.tt(c0_, T4, "add")
                SN = em.tt(em.act(Nr, AF.Square), em.act(Ni, AF.Square), "add")
                SD = em.tt(em.act(Dr, AF.Square), em.act(Di, AF.Square), "add")
                R = em.tt(SN, em.recip(SD), "mult")
                nc.sync.dma_start(out_d[:, sl], R[:])
    _split_waits(nc)
    return nc


# ---- host-side entry ---------------------------------------------------------

_CACHE = {}
TRACE = False          # set True (e.g. from test.py) to capture an NTFF trace
LAST_RESULT = None     # BassKernelResults of the most recent run


def _numpy_fallback(omega, eps, mu, rho):
    """Exact reference math in numpy (used only if mu != 1 shows up with a
    shape/path we did not compile for). Mirrors reference.py."""
    import numpy as np

    def poly(y, coeffs):
        acc = np.full_like(y, np.float32(coeffs[0]))
        for c2 in coeffs[1:]:
            acc = acc * y + np.float32(c2)
        return acc

    def _j0(x):
        y = x * x
        small = poly(y, J0_NUM) / poly(y, J0_DEN)
        z = np.float32(8.0) / x
        y2 = z * z
        xx = x - np.float32(0.785398164)
        big = np.sqrt(np.float32(TWO_OVER_PI) / x) * (
            np.cos(xx) * poly(y2, P0C) - z * np.sin(xx) * poly(y2, [c / 8 for c in Q0C]))
        return np.where(x < 8.0, small, big).astype(np.float32)

    def _j1(x):
        y = x * x
        small = x * poly(y, J1_NUM) / poly(y, J1_DEN)
        z = np.float32(8.0) / x
        y2 = z * z
        xx = x - np.float32(2.356194491)
        big = np.sqrt(np.float32(TWO_OVER_PI) / x) * (
            np.cos(xx) * poly(y2, P1C) - z * np.sin(xx) * poly(y2, [c / 8 for c in Q1C]))
        return np.where(x < 8.0, small, big).astype(np.float32)

    def _y0(x):
        y = x * x
        small = poly(y, Y0_NUM) / poly(y, Y0_DEN) + \
            np.float32(TWO_OVER_PI) * _j0(x) * np.log(x)
        z = np.float32(8.0) / x
        y2 = z * z
        xx = x - np.float32(0.785398164)
        big = np.sqrt(np.float32(TWO_OVER_PI) / x) * (
            np.sin(xx) * poly(y2, P0C) + z * np.cos(xx) * poly(y2, [c / 8 for c in Q0C]))
        return np.where(x < 8.0, small, big).astype(np.float32)

    def _y1(x):
        y = x * x
        small = x * poly(y, Y1_NUM) / poly(y, Y1_DEN) + \
            np.float32(TWO_OVER_PI) * (_j1(x) * np.log(x) - 1.0 / x)
        z = np.float32(8.0) / x
        y2 = z * z
        xx = x - np.float32(2.356194491)
        big = np.sqrt(np.float32(TWO_OVER_PI) / x) * (
            np.sin(xx) * poly(y2, P1C) + z * np.cos(xx) * poly(y2, [c / 8 for c in Q1C]))
        return np.where(x < 8.0, small, big).astype(np.float32)

    omega = omega.astype(np.float32)
    eps = eps.astype(np.float32)
    mu = mu.astype(np.float32)
    k = omega[None, :] * np.sqrt(eps * mu)
    p = np.sqrt(eps / mu)

    def tmat(kl, pl, r0, r1):
        x0 = kl * np.float32(r0)
        x1 = kl * np.float32(r1)
        j_a, y_a = _j0(x0), _y0(x0)
        j_b, y_b = _j0(x1), _y0(x1)
        jd_a, yd_a = -_j1(x0), -_y1(x0)
        jd_b, yd_b = -_j1(x1), -_y1(x1)
        pref = np.float32(PI / 2) * x0
        m00 = (pref * (yd_a * j_b - jd_a * y_b)).astype(np.complex64)
        m01 = (1j / pl) * pref * (j_a * y_b - y_a * j_b)
        m10 = (-1j * pl) * pref * (yd_a * jd_b - jd_a * yd_b)
        m11 = (pref * (j_a * yd_b - y_a * jd_b)).astype(np.complex64)
        return m00, m01, m10, m11

    M00, M01, M10, M11 = tmat(k[1], p[1], rho[1, 0], rho[1, 1])
    for l in range(2, L - 1):
        a, b, c, d = tmat(k[l], p[l], rho[l, 0], rho[l, 1])
        M00, M01, M10, M11 = (M00 * a + M01 * c, M00 * b + M01 * d,
                              M10 * a + M11 * c, M10 * b + M11 * d)

    def cfacs(z):
        j0v, j1v, y0v, y1v = _j0(z), _j1(z), _y0(z), _y1(z)
        c1 = -(j1v + 1j * y1v) / (j0v + 1j * y0v)
        c2 = -(j1v - 1j * y1v) / (j0v - 1j * y0v)
        return c1, c2

    c0_1, c0_2 = cfacs(k[0] * np.float32(rho[0, 1]))
    _, c1_2 = cfacs(k[L - 1] * np.float32(rho[L - 1, 0]))
    p0, p1 = p[0], p[L - 1]
    num = M10 + 1j * p0 * c0_2 * M00 \
        - 1j * p1 * c1_2 * (M11 + 1j * p0 * c0_2 * M01)
    den = -1j * p0 * c0_1 * M00 - M10 \
        - 1j * p1 * c1_2 * (-1j * p0 * c0_1 * M01 - M11)
    r = num / den
    return (r * np.conj(r)).real.astype(np.float32)


def kernel(omega, eps, mu, rho):
    from concourse.bass_utils import run_bass_kernel_spmd

    omega = np.ascontiguousarray(omega, dtype=np.float32)
    eps = np.ascontiguousarray(eps, dtype=np.float32)
    mu = np.ascontiguousarray(mu, dtype=np.float32)
    rho = np.asarray(rho, dtype=np.float32)
    assert omega.shape == (W,) and eps.shape == (L, W)

    mu_is_one = bool(np.all(mu == 1.0))
    if mu_is_one:
        epsk = eps
        epsp = None
    else:
        epsk = (eps * mu).astype(np.float32)
        epsp = (eps / mu).astype(np.float32)

    # The M/theta fit covers x in [XFIT_MIN, XFIT_MAX]; verify every Bessel
    # argument this input set produces lies inside, else use the NR variant.
    kmax = (omega.max() * np.sqrt(epsk.max(axis=1)))
    kmin = (omega.min() * np.sqrt(epsk.min(axis=1)))
    r64 = rho.astype(np.float64)
    xlo, xhi = np.inf, 0.0
    for l in range(1, L - 1):
        for rr in (r64[l, 0], r64[l, 1]):
            xlo = min(xlo, kmin[l] * rr)
            xhi = max(xhi, kmax[l] * rr)
    xlo = min(xlo, kmin[0] * r64[0, 1], kmin[L - 1] * r64[L - 1, 0])
    xhi = max(xhi, kmax[0] * r64[0, 1], kmax[L - 1] * r64[L - 1, 0])
    variant = "delta" if (xlo > XFIT_MIN and xhi < XFIT_MAX) else "nr"

    key = (tuple(np.asarray(rho, dtype=np.float32).ravel().tolist()),
           not mu_is_one, variant)
    if key not in _CACHE:
        _CACHE[key] = build(np.asarray(rho, dtype=np.float64), not mu_is_one,
                            variant)
    nc = _CACHE[key]

    in_maps = []
    for i in range(NCORES):
        sl = slice(i * WS, (i + 1) * WS)
        m = {"omega": omega[sl].reshape(P, FT),
             "epsk": epsk[:, sl].reshape(L, P, FT)}
        if not mu_is_one:
            m["epsp"] = epsp[:, sl].reshape(L, P, FT)
        in_maps.append(m)

    res = run_bass_kernel_spmd(nc, in_maps, core_ids=list(range(NCORES)),
                               trace=TRACE)
    global LAST_RESULT
    LAST_RESULT = res
    out = np.empty((W,), dtype=np.float32)
    for i in range(NCORES):
        out[i * WS:(i + 1) * WS] = res.results[i]["out"].reshape(WS)
    return out



# revision 7
# speedup vs baseline: 6.6680x; 6.6680x over previous
"""Trainium2 Bass kernel for the annular photonic transfer-matrix reflectance
sweep (W=2097152 wavelengths, L=6 layers), data-parallel over 8 NeuronCores.

Formulation (validated host-side to rel_l2 ~1.6e-3 vs the jax reference):
- Each shell's 2x2 transfer matrix entries are written as
      a = Ca(t)cosD - Sa(t)sinD,   b = (Cb sinD + Sb cosD)/p,
      c = (Cc sinD + Sc cosD)*p,   d = Cd cosD - Sd sinD,
  with t = omega*sqrt(eps), D = (r1-r0)*t, p = sqrt(eps).  The 8 smooth
  C/S product functions (Bessel amplitude/phase combinations at x0=t*r0,
  x1=t*r1) are fitted per shell by QUADRATICS in v = 1/t at build time
  (the reference's own y1 has a ~1e-3 jump at x=8, so degree>2 buys
  nothing).  Each quadratic is evaluated as A*(v+B)^2 + C: one ACT Square
  (free affine bias) + one DVE tensor_scalar.
- sinD/cosD: reduce t mod pi/c in ONE scalar_tensor_tensor (k<=14 so the
  single-constant reduction error is ~2e-6 rad), then ACT Sin with the
  *c fold in its free scale; the dropped (-1)^k sign scales the whole
  shell matrix and cancels in R = |num/den|^2.
- All smooth math runs in fp16 (DVE 2x/4x modes); t and the range
  reduction stay fp32.  Boundary C(z) factors are fitted the same way.
- mu == 1 per the spec; a numpy fallback guards other inputs.
"""
import numpy as np

import bass_rust
import concourse.bass as bass
import concourse.tile as tile
from concourse import mybir
from concourse.vector_clock import ScopedClock

F32 = mybir.dt.float32
F16 = mybir.dt.float16
AL = mybir.AluOpType
AF = mybir.ActivationFunctionType

W = 2097152
L = 6
NCORES = 8
P = 128
WS = W // NCORES          # 262144 elements per core
FT = WS // P              # 2048 free dim per core
FC = 1024                 # free-dim chunk per pass (2 passes)

PI = float(np.pi)
TWO_OVER_PI = 0.636619772
MAGIC = 8388608.0

# ---- NR Bessel coefficients (reference's own formulas, fp64 host eval) ------

J0_NUM = [-184.9052456, 77392.33017, -11214424.18, 651619640.7,
          -13362590354.0, 57568490574.0]
J0_DEN = [1.0, 267.8532712, 59272.64853, 9494680.718,
          1029532985.0, 57568490411.0]
J1_NUM = [-30.16036606, 15704.48260, -2972611.439, 242396853.1,
          -7895059235.0, 72362614232.0]
J1_DEN = [1.0, 376.9991397, 99447.43394, 18583304.74,
          2300535178.0, 144725228442.0]
Y0_NUM = [228.4622733, -86327.92757, 10879881.29, -512359803.6,
          7062834065.0, -2957821389.0]
Y0_DEN = [1.0, 226.1030244, 47447.26470, 7189466.438,
          745249964.8, 40076544269.0]
Y1_NUM = [8.511937935e4, -4.237922726e7, 7.349264551e9,
          -5.153438139e11, 1.275274390e13, -4.900604943e13]
Y1_DEN = [1.0, 3.549632885e3, 1.020426050e6, 2.245904002e8,
          3.733650367e10, 4.244419664e12, 2.499580570e14]
P0C = [0.2093887211e-6, -0.2073370639e-5, 0.2734510407e-4,
       -0.1098628627e-2, 1.0]
Q0C = [-0.934935152e-7, 0.7621095161e-6, -0.6911147651e-5,
       0.1430488765e-3, -0.1562499995e-1]
P1C = [-0.240337019e-6, 0.2457520174e-5, -0.3516396496e-4,
       0.183105e-2, 1.0]
Q1C = [0.105787412e-6, -0.88228987e-6, 0.8449199096e-5,
       -0.2002690873e-3, 0.04687499995]


def _hpoly(y, c):
    acc = np.full_like(y, c[0])
    for v in c[1:]:
        acc = acc * y + v
    return acc


def _j0(x):
    y = x * x
    small = _hpoly(y, J0_NUM) / _hpoly(y, J0_DEN)
    z = 8.0 / x
    y2 = z * z
    xx = x - 0.785398164
    big = np.sqrt(TWO_OVER_PI / x) * (np.cos(xx) * _hpoly(y2, P0C)
                                      - z * np.sin(xx) * _hpoly(y2, Q0C))
    return np.where(x < 8.0, small, big)


def _j1(x):
    y = x * x
    small = x * _hpoly(y, J1_NUM) / _hpoly(y, J1_DEN)
    z = 8.0 / x
    y2 = z * z
    xx = x - 2.356194491
    big = np.sqrt(TWO_OVER_PI / x) * (np.cos(xx) * _hpoly(y2, P1C)
                                      - z * np.sin(xx) * _hpoly(y2, Q1C))
    return np.where(x < 8.0, small, big)


def _y0(x):
    y = x * x
    small = _hpoly(y, Y0_NUM) / _hpoly(y, Y0_DEN) \
        + TWO_OVER_PI * _j0(x) * np.log(x)
    z = 8.0 / x
    y2 = z * z
    xx = x - 0.785398164
    big = np.sqrt(TWO_OVER_PI / x) * (np.sin(xx) * _hpoly(y2, P0C)
                                      + z * np.cos(xx) * _hpoly(y2, Q0C))
    return np.where(x < 8.0, small, big)


def _y1(x):
    y = x * x
    small = x * _hpoly(y, Y1_NUM) / _hpoly(y, Y1_DEN) \
        + TWO_OVER_PI * (_j1(x) * np.log(x) - 1.0 / x)
    z = 8.0 / x
    y2 = z * z
    xx = x - 2.356194491
    big = np.sqrt(TWO_OVER_PI / x) * (np.sin(xx) * _hpoly(y2, P1C)
                                      + z * np.cos(xx) * _hpoly(y2, Q1C))
    return np.where(x < 8.0, small, big)


def _mphi(x):
    amp = np.sqrt(PI * x / 2.0)
    j0n, y0n = _j0(x) * amp, _y0(x) * amp
    j1n, y1n = _j1(x) * amp, _y1(x) * amp
    psi = x - PI / 4
    m0 = np.hypot(j0n, y0n)
    ph0 = np.angle(np.exp(1j * (np.arctan2(y0n, j0n) - psi)))
    m1 = np.hypot(j1n, y1n)
    ph1 = np.angle(np.exp(1j * (np.arctan2(j1n, -y1n) - psi)))
    return m0, ph0, m1, ph1


def _shell_funcs(t, r0, r1):
    m0a, f0a, m1a, f1a = _mphi(t * r0)
    m0b, f0b, m1b, f1b = _mphi(t * r1)
    return (m1a * m0b * np.cos(f0b - f1a), m1a * m0b * np.sin(f0b - f1a),
            m0a * m0b * np.cos(f0b - f0a), m0a * m0b * np.sin(f0b - f0a),
            m1a * m1b * np.cos(f1b - f1a), m1a * m1b * np.sin(f1b - f1a),
            m0a * m1b * np.cos(f1b - f0a), m0a * m1b * np.sin(f1b - f0a))


def _bound_funcs(t, r):
    m0, ph0, m1, ph1 = _mphi(t * r)
    d = ph1 - ph0
    rm = m1 / m0
    return rm * np.sin(d), rm * np.cos(d)


def _fit_quad(f, lo, hi, n=3000):
    """Least-squares quadratic fit on [lo,hi]; returns square-form (A,B,C)
    for A*(v+B)^2 + C."""
    k = np.arange(n)
    x = lo + (hi - lo) * 0.5 * (1 - np.cos(np.pi * (k + 0.5) / n))
    y = f(x)
    ch = np.polynomial.chebyshev.Chebyshev.fit(x, y, 2, domain=[lo, hi])
    c2, c1, c0 = ch.convert(kind=np.polynomial.Polynomial).coef
    # guard near-linear: keep B bounded by inflating |c0|
    if abs(c0) < 1e-9:
        c0 = 1e-9 if c0 >= 0 else -1e-9
    A = c0
    B = c1 / (2 * c0)
    C = c2 - c1 * c1 / (4 * c0)
    return float(A), float(B), float(C)


def _build_fits(rho, tlo, thi):
    """rho: [L,2] float64; tlo/thi: per-layer t bounds. Returns dict."""
    fits = {}
    for l in range(1, L - 1):
        lo, hi = 1.0 / thi[l], 1.0 / tlo[l]
        r0, r1 = float(rho[l, 0]), float(rho[l, 1])
        for i, nm in enumerate(["Ca", "Sa", "Cb", "Sb", "Cc", "Sc",
                                "Cd", "Sd"]):
            fits[(l, nm)] = _fit_quad(
                lambda v, i=i: _shell_funcs(1.0 / v, r0, r1)[i], lo, hi)
    for (l, rr, pre) in [(0, float(rho[0, 1]), "b0"),
                         (L - 1, float(rho[L - 1, 0]), "b1")]:
        lo, hi = 1.0 / thi[l], 1.0 / tlo[l]
        for i, sfx in enumerate(["re", "im"]):
            fits[(l, pre + sfx)] = _fit_quad(
                lambda v, i=i: _bound_funcs(1.0 / v, rr)[i], lo, hi)
    return fits


# ---- walrus 1-sync-wait-per-instruction workaround --------------------------
_MAXW = 1


def _split_waits(nc):
    for f in nc.m.functions:
        for bb in f.blocks:
            arr = list(bb.instructions)
            out = []
            changed = False
            for mi in arr:
                si = mi.sync_info
                waits = list(si.on_wait) if si is not None and si.on_wait else []
                if len(waits) > _MAXW:
                    changed = True
                    upd = list(si.on_update) if si is not None and si.on_update \
                        else []
                    rest = waits[_MAXW:]
                    for i in range(0, len(rest), _MAXW):
                        ev = nc.engines[mi.engine].nop()
                        cur = nc.cur_bb.bb
                        cur.instructions = [
                            x for x in cur.instructions if x.name != ev.ins.name
                        ]
                        ev.ins.sync_info = bass_rust.SyncInfo(
                            on_wait=rest[i:i + _MAXW], on_update=[])
                        out.append(ev.ins)
                    mi.sync_info = bass_rust.SyncInfo(on_wait=waits[:_MAXW],
                                                      on_update=upd)
                out.append(mi)
            if changed:
                bb.instructions = out


def _patched_drain_and_barrier(self, tick_clock, wait_clock):
    nc = self.nc
    drain_inst = nc.sync.drain()
    wait_clock.add_sem_waits(
        drain_inst.ins, ScopedClock({None: tick_clock.global_clock})
    )
    nc.all_engine_barrier()
    assert self.sems is not None
    popped = nc._tile_sem_poison_stack.pop()
    assert popped is self._sem_poison
    nc.clear_and_free_semaphores(list(self.sems.allocated().values()))
    nc.all_engine_barrier()


tile.TileContext._drain_and_barrier = _patched_drain_and_barrier


def _register_const(nc, *values):
    for v in values:
        v = float(v)
        if (F32, v) in nc.const_aps.aps:
            continue
        t = nc.alloc_sbuf_tensor(f"const-f32-{v}", [128, 1], F32)
        nc.gpsimd.memset(t.ap(), v)
        nc.const_aps.aps[(F32, v)] = t.ap()
    nc.all_engine_barrier()


# ---- kernel emitter ---------------------------------------------------------

SHELL_FN = ["Ca", "Sa", "Cb", "Sb", "Cc", "Sc", "Cd", "Sd"]


def build(rho64, fits):
    nc = bass.Bass()
    # pre-register every ACT bias constant
    biases = {float(np.float32(B)) for (_, B, _) in fits.values()}
    _register_const(nc, 0.0, PI / 2, *sorted(biases))

    om_d = nc.declare_dram_parameter("omega", [P, FT], F32, isOutput=False)
    ep_d = nc.declare_dram_parameter("eps", [L, P, FT], F32, isOutput=False)
    out_d = nc.declare_dram_parameter("out", [P, FT], F32, isOutput=True)

    with tile.TileContext(nc) as tc:
        with tc.tile_pool(name="work", bufs=1) as pool:
            n = [0]

            def mk(dt, tag, bufs):
                n[0] += 1
                return pool.tile([P, FC], dt, name=f"t{n[0]}", tag=tag,
                                 bufs=bufs)

            def w32(tag="g32", bufs=6):
                return mk(F32, tag, bufs)

            def w16(tag="g16", bufs=12):
                return mk(F16, tag, bufs)

            def act(out, in_, fn, bias=0.0, scale=1.0):
                nc.scalar.activation(out[:], in_[:], fn, float(bias),
                                     float(scale))
                return out

            def vts(out, a, s1, s2=None, op0="mult", op1="add"):
                if s2 is None:
                    nc.vector.tensor_scalar(out[:], a[:], float(s1), None,
                                            AL[op0])
                else:
                    nc.vector.tensor_scalar(out[:], a[:], float(s1),
                                            float(s2), AL[op0], AL[op1])
                return out

            def tt(out, a, b, op, eng=None):
                (eng or nc.vector).tensor_tensor(out[:], a[:], b[:], AL[op])
                return out

            def stt(out, a, s, b, op0="mult", op1="add"):
                nc.vector.scalar_tensor_tensor(out[:], a[:], float(s), b[:],
                                               AL[op0], AL[op1])
                return out

            for ci in range(FT // FC):
                sl = slice(ci * FC, (ci + 1) * FC)
                # -------- loads, sqrt, t, v --------
                omega = w32(tag="om", bufs=2)
                nc.sync.dma_start(omega[:], om_d[:, sl])
                om16 = act(w16(tag="om16", bufs=2), omega, AF.Copy)
                sq, p16, t_ = [], [], []
                for l in range(L):
                    e = w32(tag="eps", bufs=3)
                    nc.sync.dma_start(e[:], ep_d[l, :, sl])
                    sq.append(act(w32(tag="sq", bufs=3), e, AF.Sqrt))
                    p16.append(act(w16(tag="p16", bufs=8), sq[l], AF.Copy))
                    t_.append(tt(w32(tag="t", bufs=6), omega, sq[l], "mult"))
                # ACT ln/exp block (one table set)
                v16 = []
                for l in range(L):
                    ln = act(w32(tag="ln", bufs=2), t_[l], AF.Ln)
                    v16.append(act(w16(tag="v16", bufs=8), ln, AF.Exp,
                                   0.0, -1.0))

                # -------- boundaries (early: frees v16[0], v16[5]) --------
                def boundary(l, pre):
                    Ar, Br, Cr = fits[(l, pre + "re")]
                    Ai, Bi, Ci = fits[(l, pre + "im")]
                    qre = act(w32(), v16[l], AF.Square, float(np.float32(Br)))
                    cre = vts(w16(), qre, Ar, Cr)
                    qim = act(w16(), v16[l], AF.Square, float(np.float32(Bi)))
                    cim = vts(w16(), qim, Ai, Ci)
                    ur = tt(w16(tag="bnd", bufs=8), p16[l], cre, "mult",
                            nc.gpsimd)
                    ui = tt(w16(tag="bnd", bufs=8), p16[l], cim, "mult",
                            nc.gpsimd)
                    return ur, ui

                ur0, ui0 = boundary(0, "b0")
                vr0, vi0 = boundary(L - 1, "b1")

                # -------- shells + sequential chain --------
                def shell(l):
                    r0 = float(rho64[l, 0])
                    r1 = float(rho64[l, 1])
                    c = float(np.float32(np.float64(r1) - np.float64(r0)))
                    cpi = float(np.float32(np.float64(c) / np.pi))
                    pic = float(np.float32(np.pi / np.float64(c)))
                    tr = vts(w32(), t_[l], cpi)
                    kf = vts(w32(), tr, MAGIC, MAGIC, "add", "subtract")
                    xr = stt(w32(), kf, -pic, t_[l])
                    SD = act(w16(tag="sdcd", bufs=4), xr, AF.Sin, 0.0, c)
                    CD = act(w16(tag="sdcd", bufs=4), xr, AF.Sin, PI / 2, -c)
                    Pv = {}
                    for nm in SHELL_FN:
                        A, B, C = fits[(l, nm)]
                        B = float(np.float32(B))
                        if nm.startswith("C"):
                            q = act(w16(), v16[l], AF.Square, B)
                        else:
                            q = act(w32(), v16[l], AF.Square, B)
                        Pv[nm] = vts(w16(), q, A, C)
                    TCa = tt(w16(), Pv["Ca"], CD, "mult", nc.gpsimd)
                    TSa = tt(w16(), Pv["Sa"], SD, "mult", nc.gpsimd)
                    TCb = tt(w16(), Pv["Cb"], SD, "mult", nc.gpsimd)
                    TSb = tt(w16(), Pv["Sb"], CD, "mult", nc.gpsimd)
                    TCc = tt(w16(), Pv["Cc"], SD, "mult")
                    TSc = tt(w16(), Pv["Sc"], CD, "mult")
                    TCd = tt(w16(), Pv["Cd"], CD, "mult")
                    TSd = tt(w16(), Pv["Sd"], SD, "mult")
                    a = tt(w16(tag="mm", bufs=12), TCa, TSa, "subtract")
                    beta = tt(w16(), TCb, TSb, "add")
                    gam = tt(w16(), TCc, TSc, "add")
                    d = tt(w16(tag="mm", bufs=12), TCd, TSd, "subtract")
                    rp = tt(w16(), om16, v16[l], "mult")
                    b = tt(w16(tag="mm", bufs=12), beta, rp, "mult")
                    cc = tt(w16(tag="mm", bufs=12), gam, p16[l], "mult")
                    return a, b, cc, d

                def join(Mx, My):
                    a1, b1, c1, d1 = Mx
                    a2, b2, c2, d2 = My
                    A = tt(w16(tag="mm", bufs=12),
                           tt(w16(), a1, a2, "mult"),
                           tt(w16(), b1, c2, "mult"), "subtract")
                    Bq = tt(w16(tag="mm", bufs=12),
                            tt(w16(), a1, b2, "mult"),
                            tt(w16(), b1, d2, "mult"), "add")
                    C = tt(w16(tag="mm", bufs=12),
                           tt(w16(), c1, a2, "mult"),
                           tt(w16(), d1, c2, "mult"), "add")
                    D = tt(w16(tag="mm", bufs=12),
                           tt(w16(), d1, d2, "mult"),
                           tt(w16(), c1, b2, "mult"), "subtract")
                    return A, Bq, C, D

                M = shell(1)
                for l in range(2, L - 1):
                    M = join(M, shell(l))
                A, B, C, D = M

                # -------- assembly (baseline formula) --------
                Q = tt(w16(), ui0, B, "mult")
                er = tt(w16(), D, tt(w16(), ur0, B, "mult"), "add")
                T1 = tt(w16(), vi0, Q, "mult")
                T2 = tt(w16(), vr0, er, "mult")
                T3 = tt(w16(), vr0, Q, "mult")
                T4 = tt(w16(), vi0, er, "mult")
                aAr = tt(w16(), ur0, A, "mult")
                aAi = tt(w16(), ui0, A, "mult")
                b0 = tt(w16(), C, aAr, "subtract")
                b1 = tt(w16(), b0, T2, "add")
                Nr = tt(w16(), b1, T1, "subtract")
                Dr = tt(w16(), b1, T1, "add")
                c0_ = tt(w16(), aAi, T3, "subtract")
                Ni = tt(w16(), c0_, T4, "subtract")
                Di = tt(w16(), c0_, T4, "add")
                SN = tt(w16(), tt(w16(), Nr, Nr, "mult"),
                        tt(w16(), Ni, Ni, "mult"), "add")
                SDn = tt(w16(), tt(w16(), Dr, Dr, "mult"),
                         tt(w16(), Di, Di, "mult"), "add")
                lnD = act(w32(), SDn, AF.Ln)
                rec = act(w32(), lnD, AF.Exp, 0.0, -1.0)
                R = tt(w32(), SN, rec, "mult")
                nc.sync.dma_start(out_d[:, sl], R[:])
    _split_waits(nc)
    return nc


# ---- host-side entry --------------------------------------------------------

_CACHE = {}
TRACE = False
LAST_RESULT = None


def _numpy_ref(omega, eps, mu, rho):
    """Exact reference math in numpy (fallback for mu != 1)."""
    omega = omega.astype(np.float64)
    eps = eps.astype(np.float64)
    mu = mu.astype(np.float64)
    rho = rho.astype(np.float64)
    k = omega[None, :] * np.sqrt(eps * mu)
    p = np.sqrt(eps / mu)

    def tmat(kl, pl, r0, r1):
        x0, x1 = kl * r0, kl * r1
        j_a, y_a = _j0(x0), _y0(x0)
        j_b, y_b = _j0(x1), _y0(x1)
        jd_a, yd_a = -_j1(x0), -_y1(x0)
        jd_b, yd_b = -_j1(x1), -_y1(x1)
        pref = (PI / 2) * x0
        m00 = pref * (yd_a * j_b - jd_a * y_b)
        m01 = (1j / pl) * pref * (j_a * y_b - y_a * j_b)
        m10 = (-1j * pl) * pref * (yd_a * jd_b - jd_a * yd_b)
        m11 = pref * (j_a * yd_b - y_a * jd_b)
        return m00 + 0j, m01, m10, m11 + 0j

    M00, M01, M10, M11 = tmat(k[1], p[1], rho[1, 0], rho[1, 1])
    for l in range(2, L - 1):
        a, b, c, d = tmat(k[l], p[l], rho[l, 0], rho[l, 1])
        M00, M01, M10, M11 = (M00 * a + M01 * c, M00 * b + M01 * d,
                              M10 * a + M11 * c, M10 * b + M11 * d)

    def cfacs(z):
        j0v, j1v, y0v, y1v = _j0(z), _j1(z), _y0(z), _y1(z)
        c1 = -(j1v + 1j * y1v) / (j0v + 1j * y0v)
        c2 = -(j1v - 1j * y1v) / (j0v - 1j * y0v)
        return c1, c2

    c0_1, c0_2 = cfacs(k[0] * rho[0, 1])
    _, c1_2 = cfacs(k[L - 1] * rho[L - 1, 0])
    p0, p1 = p[0], p[L - 1]
    num = M10 + 1j * p0 * c0_2 * M00 \
        - 1j * p1 * c1_2 * (M11 + 1j * p0 * c0_2 * M01)
    den = -1j * p0 * c0_1 * M00 - M10 \
        - 1j * p1 * c1_2 * (-1j * p0 * c0_1 * M01 - M11)
    r = num / den
    return (r * np.conj(r)).real.astype(np.float32)


def kernel(omega, eps, mu, rho):
    from concourse.bass_utils import run_bass_kernel_spmd

    omega = np.ascontiguousarray(omega, dtype=np.float32)
    eps = np.ascontiguousarray(eps, dtype=np.float32)
    mu = np.ascontiguousarray(mu, dtype=np.float32)
    rho = np.asarray(rho, dtype=np.float32)
    assert omega.shape == (W,) and eps.shape == (L, W)

    if not bool(np.all(mu == 1.0)):
        return _numpy_ref(omega, eps, mu, rho)

    rho64 = rho.astype(np.float64)
    om_lo, om_hi = float(omega.min()), float(omega.max())
    e_lo = eps.min(axis=1).astype(np.float64)
    e_hi = eps.max(axis=1).astype(np.float64)
    tlo = om_lo * np.sqrt(e_lo) * 0.999
    thi = om_hi * np.sqrt(e_hi) * 1.001

    key = (rho.tobytes(),
           tuple(np.round(tlo, 3).tolist()), tuple(np.round(thi, 3).tolist()))
    if key not in _CACHE:
        fits = _build_fits(rho64, tlo, thi)
        _CACHE[key] = build(rho64, fits)
    nc = _CACHE[key]

    in_maps = []
    for i in range(NCORES):
        sl = slice(i * WS, (i + 1) * WS)
        in_maps.append({"omega": omega[sl].reshape(P, FT),
                        "eps": eps[:, sl].reshape(L, P, FT)})

    res = run_bass_kernel_spmd(nc, in_maps, core_ids=list(range(NCORES)),
                               trace=TRACE)
    global LAST_RESULT
    LAST_RESULT = res
    out = np.empty((W,), dtype=np.float32)
    for i in range(NCORES):
        out[i * WS:(i + 1) * WS] = res.results[i]["out"].reshape(WS)
    return out


# revision 30
# speedup vs baseline: 8.5644x; 1.2844x over previous
"""Trainium2 Bass kernel for the annular photonic transfer-matrix reflectance
sweep (W=2097152 wavelengths, L=6 layers), data-parallel over 8 NeuronCores.

Formulation (validated host-side to rel_l2 ~1.6e-3 vs the jax reference):
- Each shell's 2x2 transfer matrix entries are written as
      a = Ca(t)cosD - Sa(t)sinD,   b = (Cb sinD + Sb cosD)/p,
      c = (Cc sinD + Sc cosD)*p,   d = Cd cosD - Sd sinD,
  with t = omega*sqrt(eps), D = (r1-r0)*t, p = sqrt(eps).  The 8 smooth
  C/S product functions (Bessel amplitude/phase combinations at x0=t*r0,
  x1=t*r1) are fitted per shell by QUADRATICS in v = 1/t at build time
  (the reference's own y1 has a ~1e-3 jump at x=8, so degree>2 buys
  nothing).  Each quadratic is evaluated as A*(v+B)^2 + C: one ACT Square
  (free affine bias) + one DVE tensor_scalar.
- sinD/cosD: reduce t mod pi/c in ONE scalar_tensor_tensor (k<=14 so the
  single-constant reduction error is ~2e-6 rad), then ACT Sin with the
  *c fold in its free scale; the dropped (-1)^k sign scales the whole
  shell matrix and cancels in R = |num/den|^2.
- All smooth math runs in fp16 (DVE 2x/4x modes); t and the range
  reduction stay fp32.  Boundary C(z) factors are fitted the same way.
- mu == 1 per the spec; a numpy fallback guards other inputs.
"""
import numpy as np

import bass_rust
import concourse.bass as bass
import concourse.tile as tile
from concourse import mybir
from concourse.vector_clock import ScopedClock

F32 = mybir.dt.float32
F16 = mybir.dt.float16
AL = mybir.AluOpType
AF = mybir.ActivationFunctionType

W = 2097152
L = 6
NCORES = 8
P = 128
WS = W // NCORES          # 262144 elements per core
FT = WS // P              # 2048 free dim per core
FC = 1024                 # free-dim chunk per pass (2 passes)

PI = float(np.pi)
TWO_OVER_PI = 0.636619772
MAGIC = 8388608.0

# ---- NR Bessel coefficients (reference's own formulas, fp64 host eval) ------

J0_NUM = [-184.9052456, 77392.33017, -11214424.18, 651619640.7,
          -13362590354.0, 57568490574.0]
J0_DEN = [1.0, 267.8532712, 59272.64853, 9494680.718,
          1029532985.0, 57568490411.0]
J1_NUM = [-30.16036606, 15704.48260, -2972611.439, 242396853.1,
          -7895059235.0, 72362614232.0]
J1_DEN = [1.0, 376.9991397, 99447.43394, 18583304.74,
          2300535178.0, 144725228442.0]
Y0_NUM = [228.4622733, -86327.92757, 10879881.29, -512359803.6,
          7062834065.0, -2957821389.0]
Y0_DEN = [1.0, 226.1030244, 47447.26470, 7189466.438,
          745249964.8, 40076544269.0]
Y1_NUM = [8.511937935e4, -4.237922726e7, 7.349264551e9,
          -5.153438139e11, 1.275274390e13, -4.900604943e13]
Y1_DEN = [1.0, 3.549632885e3, 1.020426050e6, 2.245904002e8,
          3.733650367e10, 4.244419664e12, 2.499580570e14]
P0C = [0.2093887211e-6, -0.2073370639e-5, 0.2734510407e-4,
       -0.1098628627e-2, 1.0]
Q0C = [-0.934935152e-7, 0.7621095161e-6, -0.6911147651e-5,
       0.1430488765e-3, -0.1562499995e-1]
P1C = [-0.240337019e-6, 0.2457520174e-5, -0.3516396496e-4,
       0.183105e-2, 1.0]
Q1C = [0.105787412e-6, -0.88228987e-6, 0.8449199096e-5,
       -0.2002690873e-3, 0.04687499995]


def _hpoly(y, c):
    acc = np.full_like(y, c[0])
    for v in c[1:]:
        acc = acc * y + v
    return acc


def _j0(x):
    y = x * x
    small = _hpoly(y, J0_NUM) / _hpoly(y, J0_DEN)
    z = 8.0 / x
    y2 = z * z
    xx = x - 0.785398164
    big = np.sqrt(TWO_OVER_PI / x) * (np.cos(xx) * _hpoly(y2, P0C)
                                      - z * np.sin(xx) * _hpoly(y2, Q0C))
    return np.where(x < 8.0, small, big)


def _j1(x):
    y = x * x
    small = x * _hpoly(y, J1_NUM) / _hpoly(y, J1_DEN)
    z = 8.0 / x
    y2 = z * z
    xx = x - 2.356194491
    big = np.sqrt(TWO_OVER_PI / x) * (np.cos(xx) * _hpoly(y2, P1C)
                                      - z * np.sin(xx) * _hpoly(y2, Q1C))
    return np.where(x < 8.0, small, big)


def _y0(x):
    y = x * x
    small = _hpoly(y, Y0_NUM) / _hpoly(y, Y0_DEN) \
        + TWO_OVER_PI * _j0(x) * np.log(x)
    z = 8.0 / x
    y2 = z * z
    xx = x - 0.785398164
    big = np.sqrt(TWO_OVER_PI / x) * (np.sin(xx) * _hpoly(y2, P0C)
                                      + z * np.cos(xx) * _hpoly(y2, Q0C))
    return np.where(x < 8.0, small, big)


def _y1(x):
    y = x * x
    small = x * _hpoly(y, Y1_NUM) / _hpoly(y, Y1_DEN) \
        + TWO_OVER_PI * (_j1(x) * np.log(x) - 1.0 / x)
    z = 8.0 / x
    y2 = z * z
    xx = x - 2.356194491
    big = np.sqrt(TWO_OVER_PI / x) * (np.sin(xx) * _hpoly(y2, P1C)
                                      + z * np.cos(xx) * _hpoly(y2, Q1C))
    return np.where(x < 8.0, small, big)


def _mphi(x):
    amp = np.sqrt(PI * x / 2.0)
    j0n, y0n = _j0(x) * amp, _y0(x) * amp
    j1n, y1n = _j1(x) * amp, _y1(x) * amp
    psi = x - PI / 4
    m0 = np.hypot(j0n, y0n)
    ph0 = np.angle(np.exp(1j * (np.arctan2(y0n, j0n) - psi)))
    m1 = np.hypot(j1n, y1n)
    ph1 = np.angle(np.exp(1j * (np.arctan2(j1n, -y1n) - psi)))
    return m0, ph0, m1, ph1


def _shell_funcs(t, r0, r1):
    m0a, f0a, m1a, f1a = _mphi(t * r0)
    m0b, f0b, m1b, f1b = _mphi(t * r1)
    return (m1a * m0b * np.cos(f0b - f1a), m1a * m0b * np.sin(f0b - f1a),
            m0a * m0b * np.cos(f0b - f0a), m0a * m0b * np.sin(f0b - f0a),
            m1a * m1b * np.cos(f1b - f1a), m1a * m1b * np.sin(f1b - f1a),
            m0a * m1b * np.cos(f1b - f0a), m0a * m1b * np.sin(f1b - f0a))


def _bound_funcs(t, r):
    m0, ph0, m1, ph1 = _mphi(t * r)
    d = ph1 - ph0
    rm = m1 / m0
    return rm * np.sin(d), rm * np.cos(d)


LIN_TOL = 2.5e-3


def _fit_quad(f, lo, hi, n=3000):
    """Fit f on [lo,hi]: linear if it reaches LIN_TOL, else quadratic in
    square-form ("quad", A, B, C) for A*(v+B)^2 + C."""
    k = np.arange(n)
    x = lo + (hi - lo) * 0.5 * (1 - np.cos(np.pi * (k + 0.5) / n))
    y = f(x)
    ch1 = np.polynomial.chebyshev.Chebyshev.fit(x, y, 1, domain=[lo, hi])
    if np.abs(ch1(x) - y).max() < LIN_TOL:
        c1, c0 = 0.0, 0.0
        co = ch1.convert(kind=np.polynomial.Polynomial).coef
        c2 = co[0]
        c1 = co[1] if len(co) > 1 else 0.0
        return ("lin", float(c1), float(c2))
    ch = np.polynomial.chebyshev.Chebyshev.fit(x, y, 2, domain=[lo, hi])
    c2, c1, c0 = ch.convert(kind=np.polynomial.Polynomial).coef
    if abs(c0) < 1e-9:
        c0 = 1e-9 if c0 >= 0 else -1e-9
    return ("quad", float(c0), float(c1 / (2 * c0)),
            float(c2 - c1 * c1 / (4 * c0)))


def _build_fits(rho, tlo, thi):
    """rho: [L,2] float64; tlo/thi: per-layer t bounds. Returns dict."""
    fits = {}
    for l in range(1, L - 1):
        lo, hi = 1.0 / thi[l], 1.0 / tlo[l]
        r0, r1 = float(rho[l, 0]), float(rho[l, 1])
        for i, nm in enumerate(["Ca", "Sa", "Cb", "Sb", "Cc", "Sc",
                                "Cd", "Sd"]):
            fits[(l, nm)] = _fit_quad(
                lambda v, i=i: _shell_funcs(1.0 / v, r0, r1)[i], lo, hi)
    for (l, rr, pre) in [(0, float(rho[0, 1]), "b0"),
                         (L - 1, float(rho[L - 1, 0]), "b1")]:
        lo, hi = 1.0 / thi[l], 1.0 / tlo[l]
        for i, sfx in enumerate(["re", "im"]):
            fits[(l, pre + sfx)] = _fit_quad(
                lambda v, i=i: _bound_funcs(1.0 / v, rr)[i], lo, hi)
    return fits


# ---- walrus 1-sync-wait-per-instruction workaround --------------------------
_MAXW = 1


def _split_waits(nc):
    for f in nc.m.functions:
        for bb in f.blocks:
            arr = list(bb.instructions)
            out = []
            changed = False
            for mi in arr:
                si = mi.sync_info
                waits = list(si.on_wait) if si is not None and si.on_wait else []
                if len(waits) > _MAXW:
                    changed = True
                    upd = list(si.on_update) if si is not None and si.on_update \
                        else []
                    rest = waits[_MAXW:]
                    for i in range(0, len(rest), _MAXW):
                        ev = nc.engines[mi.engine].nop()
                        cur = nc.cur_bb.bb
                        cur.instructions = [
                            x for x in cur.instructions if x.name != ev.ins.name
                        ]
                        ev.ins.sync_info = bass_rust.SyncInfo(
                            on_wait=rest[i:i + _MAXW], on_update=[])
                        out.append(ev.ins)
                    mi.sync_info = bass_rust.SyncInfo(on_wait=waits[:_MAXW],
                                                      on_update=upd)
                out.append(mi)
            if changed:
                bb.instructions = out


def _patched_drain_and_barrier(self, tick_clock, wait_clock):
    nc = self.nc
    drain_inst = nc.sync.drain()
    wait_clock.add_sem_waits(
        drain_inst.ins, ScopedClock({None: tick_clock.global_clock})
    )
    nc.all_engine_barrier()
    assert self.sems is not None
    popped = nc._tile_sem_poison_stack.pop()
    assert popped is self._sem_poison
    nc.clear_and_free_semaphores(list(self.sems.allocated().values()))
    nc.all_engine_barrier()


tile.TileContext._drain_and_barrier = _patched_drain_and_barrier


def _register_const(nc, *values):
    for v in values:
        v = float(v)
        if (F32, v) in nc.const_aps.aps:
            continue
        t = nc.alloc_sbuf_tensor(f"const-f32-{v}", [128, 1], F32)
        nc.gpsimd.memset(t.ap(), v)
        nc.const_aps.aps[(F32, v)] = t.ap()
    nc.all_engine_barrier()


# ---- kernel emitter ---------------------------------------------------------

SHELL_FN = ["Ca", "Sa", "Cb", "Sb", "Cc", "Sc", "Cd", "Sd"]


def build(rho64, fits):
    nc = bass.Bass()
    biases = {float(np.float32(v[2])) for v in fits.values() if v[0] == "quad"}
    _register_const(nc, 0.0, PI / 2, *sorted(biases))

    om_d = nc.declare_dram_parameter("omega", [P, FT], F32, isOutput=False)
    ep_d = nc.declare_dram_parameter("eps", [L, P, FT], F32, isOutput=False)
    out_d = nc.declare_dram_parameter("out", [P, FT], F32, isOutput=True)

    with tile.TileContext(nc) as tc:
        with tc.tile_pool(name="work", bufs=1) as pool:
            n = [0]

            def mk(dt, tag, bufs):
                n[0] += 1
                return pool.tile([P, FC], dt, name=f"t{n[0]}", tag=tag,
                                 bufs=bufs)

            def w32(tag="g32", bufs=5):
                return mk(F32, tag, bufs)

            def w16(tag="g16", bufs=14):
                return mk(F16, tag, bufs)

            def act(out, in_, fn, bias=0.0, scale=1.0):
                nc.scalar.activation(out[:], in_[:], fn, float(bias),
                                     float(scale))
                return out

            def vts(out, a, s1, s2=None, op0="mult", op1="add"):
                if s2 is None:
                    nc.vector.tensor_scalar(out[:], a[:], float(s1), None,
                                            AL[op0])
                else:
                    nc.vector.tensor_scalar(out[:], a[:], float(s1),
                                            float(s2), AL[op0], AL[op1])
                return out

            def tt(out, a, b, op):
                nc.vector.tensor_tensor(out[:], a[:], b[:], AL[op])
                return out

            def stt(out, a, s, b, op0="mult", op1="add"):
                nc.vector.scalar_tensor_tensor(out[:], a[:], float(s), b[:],
                                               AL[op0], AL[op1])
                return out

            def poly16(fit, v16l):
                kind = fit[0]
                if kind == "lin":
                    return act(w16(), v16l, AF.Copy, fit[2], fit[1])
                _, A, B, C = fit
                B = float(np.float32(B))
                if abs(B) < 2.0:
                    q = act(w16(), v16l, AF.Square, B)
                    return vts(w16(), q, A, C)
                q = act(w32(), v16l, AF.Square, B)
                return act(w16(), q, AF.Copy, C, A)

            def chunk(ci, pre_tail=None):
                sl = slice(ci * FC, (ci + 1) * FC)
                omega = w32(tag="om", bufs=2)
                nc.sync.dma_start(omega[:], om_d[:, sl])
                ln_om = act(w32(tag="lnom", bufs=1), omega, AF.Ln)
                t16, v16, t_ = {}, {}, {}

                def layerA(l):
                    e = w32(tag="eps", bufs=2)
                    nc.sync.dma_start(e[:], ep_d[l, :, sl])
                    lne = act(w32(tag="sq", bufs=3), e, AF.Ln)
                    ln_t = stt(w32(tag="ln", bufs=2), lne, 0.5, ln_om)
                    t_[l] = act(w32(tag="t", bufs=4), ln_t, AF.Exp)
                    t16[l] = vts(w16(tag="t16", bufs=8), t_[l], 1.0)
                    v16[l] = act(w16(tag="v16", bufs=8), ln_t, AF.Exp,
                                 0.0, -1.0)

                def boundary(l, pre):
                    cre = poly16(fits[(l, pre + "re")], v16[l])
                    cim = poly16(fits[(l, pre + "im")], v16[l])
                    ur = tt(w16(tag="bnd", bufs=10), t16[l], cre, "mult")
                    ui = tt(w16(tag="bnd", bufs=10), t16[l], cim, "mult")
                    return ur, ui

                def shell(l):
                    r0 = float(rho64[l, 0])
                    r1 = float(rho64[l, 1])
                    c = float(np.float32(np.float64(r1) - np.float64(r0)))
                    cpi = float(np.float32(np.float64(c) / np.pi))
                    pic = float(np.float32(np.pi / np.float64(c)))
                    tr = vts(w32(), t_[l], cpi)
                    kf = vts(w32(), tr, MAGIC, MAGIC, "add", "subtract")
                    xr = stt(w32(), kf, -pic, t_[l])
                    SD = act(w16(tag="sdcd", bufs=6), xr, AF.Sin, 0.0, c)
                    CD = act(w16(tag="sdcd", bufs=6), xr, AF.Sin,
                             PI / 2, -c)
                    Pv = {nm: poly16(fits[(l, nm)], v16[l])
                          for nm in SHELL_FN}
                    TCa = tt(w16(), Pv["Ca"], CD, "mult")
                    TSa = tt(w16(), Pv["Sa"], SD, "mult")
                    TCb = tt(w16(), Pv["Cb"], SD, "mult")
                    TSb = tt(w16(), Pv["Sb"], CD, "mult")
                    TCc = tt(w16(), Pv["Cc"], SD, "mult")
                    TSc = tt(w16(), Pv["Sc"], CD, "mult")
                    TCd = tt(w16(), Pv["Cd"], CD, "mult")
                    TSd = tt(w16(), Pv["Sd"], SD, "mult")
                    a = tt(w16(tag="mm", bufs=16), TCa, TSa, "subtract")
                    beta = tt(w16(), TCb, TSb, "add")
                    gam = tt(w16(), TCc, TSc, "add")
                    d = tt(w16(tag="mm", bufs=16), TCd, TSd, "subtract")
                    b = tt(w16(tag="mm", bufs=16), beta, v16[l], "mult")
                    cc = tt(w16(tag="mm", bufs=16), gam, t16[l], "mult")
                    return a, b, cc, d

                def join(Mx, My):
                    a1, b1, c1, d1 = Mx
                    a2, b2, c2, d2 = My
                    A = tt(w16(tag="mm", bufs=16),
                           tt(w16(), a1, a2, "mult"),
                           tt(w16(), b1, c2, "mult"), "subtract")
                    Bq = tt(w16(tag="mm", bufs=16),
                            tt(w16(), a1, b2, "mult"),
                            tt(w16(), b1, d2, "mult"), "add")
                    C = tt(w16(tag="mm", bufs=16),
                           tt(w16(), c1, a2, "mult"),
                           tt(w16(), d1, c2, "mult"), "add")
                    D = tt(w16(tag="mm", bufs=16),
                           tt(w16(), d1, d2, "mult"),
                           tt(w16(), c1, b2, "mult"), "subtract")
                    return A, Bq, C, D

                layerA(0)
                layerA(L - 1)
                if pre_tail is not None:
                    pre_tail()
                layerA(1)
                u0 = boundary(0, "b0")
                u1 = boundary(L - 1, "b1")
                M = shell(1)
                layerA(2)
                layerA(3)
                M = join(M, shell(2))
                M = join(M, shell(3))
                layerA(4)
                M = join(M, shell(4))
                return dict(sl=sl, u0=u0, u1=u1, M=M)

            def tail(S):
                A, B, C, D = S["M"]
                ur0, ui0 = S["u0"]
                vr0, vi0 = S["u1"]
                Q = tt(w16(), ui0, B, "mult")
                er = tt(w16(), D, tt(w16(), ur0, B, "mult"), "add")
                T1 = tt(w16(), vi0, Q, "mult")
                T2 = tt(w16(), vr0, er, "mult")
                T3 = tt(w16(), vr0, Q, "mult")
                T4 = tt(w16(), vi0, er, "mult")
                aAr = tt(w16(), ur0, A, "mult")
                aAi = tt(w16(), ui0, A, "mult")
                b0 = tt(w16(), C, aAr, "subtract")
                b1 = tt(w16(), b0, T2, "add")
                Nr = tt(w16(), b1, T1, "subtract")
                Dr = tt(w16(), b1, T1, "add")
                c0_ = tt(w16(), aAi, T3, "subtract")
                Ni = tt(w16(), c0_, T4, "subtract")
                Di = tt(w16(), c0_, T4, "add")
                SN = tt(w32(), act(w32(), Nr, AF.Square),
                        act(w32(), Ni, AF.Square), "add")
                SDn = tt(w32(), act(w32(), Dr, AF.Square),
                         act(w32(), Di, AF.Square), "add")
                lnD = act(w32(), SDn, AF.Ln)
                rec = act(w32(), lnD, AF.Exp, 0.0, -1.0)
                R = tt(w32(), SN, rec, "mult")
                nc.sync.dma_start(out_d[:, S["sl"]], R[:])

            S0 = chunk(0)
            tail(S0)
            S1 = chunk(1)
            tail(S1)
    _split_waits(nc)
    return nc


# ---- host-side entry --------------------------------------------------------

_CACHE = {}
TRACE = False
LAST_RESULT = None


def _numpy_ref(omega, eps, mu, rho):
    """Exact reference math in numpy (fallback for mu != 1)."""
    omega = omega.astype(np.float64)
    eps = eps.astype(np.float64)
    mu = mu.astype(np.float64)
    rho = rho.astype(np.float64)
    k = omega[None, :] * np.sqrt(eps * mu)
    p = np.sqrt(eps / mu)

    def tmat(kl, pl, r0, r1):
        x0, x1 = kl * r0, kl * r1
        j_a, y_a = _j0(x0), _y0(x0)
        j_b, y_b = _j0(x1), _y0(x1)
        jd_a, yd_a = -_j1(x0), -_y1(x0)
        jd_b, yd_b = -_j1(x1), -_y1(x1)
        pref = (PI / 2) * x0
        m00 = pref * (yd_a * j_b - jd_a * y_b)
        m01 = (1j / pl) * pref * (j_a * y_b - y_a * j_b)
        m10 = (-1j * pl) * pref * (yd_a * jd_b - jd_a * yd_b)
        m11 = pref * (j_a * yd_b - y_a * jd_b)
        return m00 + 0j, m01, m10, m11 + 0j

    M00, M01, M10, M11 = tmat(k[1], p[1], rho[1, 0], rho[1, 1])
    for l in range(2, L - 1):
        a, b, c, d = tmat(k[l], p[l], rho[l, 0], rho[l, 1])
        M00, M01, M10, M11 = (M00 * a + M01 * c, M00 * b + M01 * d,
                              M10 * a + M11 * c, M10 * b + M11 * d)

    def cfacs(z):
        j0v, j1v, y0v, y1v = _j0(z), _j1(z), _y0(z), _y1(z)
        c1 = -(j1v + 1j * y1v) / (j0v + 1j * y0v)
        c2 = -(j1v - 1j * y1v) / (j0v - 1j * y0v)
        return c1, c2

    c0_1, c0_2 = cfacs(k[0] * rho[0, 1])
    _, c1_2 = cfacs(k[L - 1] * rho[L - 1, 0])
    p0, p1 = p[0], p[L - 1]
    num = M10 + 1j * p0 * c0_2 * M00 \
        - 1j * p1 * c1_2 * (M11 + 1j * p0 * c0_2 * M01)
    den = -1j * p0 * c0_1 * M00 - M10 \
        - 1j * p1 * c1_2 * (-1j * p0 * c0_1 * M01 - M11)
    r = num / den
    return (r * np.conj(r)).real.astype(np.float32)


def kernel(omega, eps, mu, rho):
    from concourse.bass_utils import run_bass_kernel_spmd

    omega = np.ascontiguousarray(omega, dtype=np.float32)
    eps = np.ascontiguousarray(eps, dtype=np.float32)
    mu = np.ascontiguousarray(mu, dtype=np.float32)
    rho = np.asarray(rho, dtype=np.float32)
    assert omega.shape == (W,) and eps.shape == (L, W)

    if not bool(np.all(mu == 1.0)):
        return _numpy_ref(omega, eps, mu, rho)

    rho64 = rho.astype(np.float64)
    om_lo, om_hi = float(omega.min()), float(omega.max())
    e_lo = eps.min(axis=1).astype(np.float64)
    e_hi = eps.max(axis=1).astype(np.float64)
    tlo = om_lo * np.sqrt(e_lo) * 0.999
    thi = om_hi * np.sqrt(e_hi) * 1.001

    key = (rho.tobytes(),
           tuple(np.round(tlo, 3).tolist()), tuple(np.round(thi, 3).tolist()))
    if key not in _CACHE:
        fits = _build_fits(rho64, tlo, thi)
        _CACHE[key] = build(rho64, fits)
    nc = _CACHE[key]

    in_maps = []
    for i in range(NCORES):
        sl = slice(i * WS, (i + 1) * WS)
        in_maps.append({"omega": omega[sl].reshape(P, FT),
                        "eps": eps[:, sl].reshape(L, P, FT)})

    res = run_bass_kernel_spmd(nc, in_maps, core_ids=list(range(NCORES)),
                               trace=TRACE)
    global LAST_RESULT
    LAST_RESULT = res
    out = np.empty((W,), dtype=np.float32)
    for i in range(NCORES):
        out[i * WS:(i + 1) * WS] = res.results[i]["out"].reshape(WS)
    return out
